# revision 106
# baseline (speedup 1.0000x reference)
"""Multi-head cross-attention Trainium2 kernel (8-core SPMD).

Sharding: 2 batch groups x 4 cores. Core c handles batch b = c // 4 and
heads [4*(c%4), 4*(c%4)+4). Each core computes its 4 heads' attention
output and a partial output projection (row-sharded Wp); the host sums
the 4 partials per batch (the all-reduce step of tensor parallelism) and
adds the constant bv @ Wp^T row (softmax weights sum to 1, so the V bias
contributes a per-core constant).

Engine plan (per core, "mh" = head pair, 2 per core):
  PE:   fp8e4 DoubleRow everywhere precision allows:
        - Q/K/V projections contract 2 k-tiles per DR instr (x/e/W fp8,
          weights scaled x32 into fp8 normal range); V lands directly in
          [s, hd] layout (lhsT = e^T k-pair), both heads per instr
        - QK^T via the zero-half trick: a custom-stride AP pairs each
          K-tile with a shared 128-col zero block, so the 64-deep
          contraction runs at DR rate
        - AV is FLIPPED: out[t,hd] = ej^T @ VA per 128-row t-subtile, so
          each matmul moves only 65 output columns (65 cols/instr vs 512
          in the [hd,t] orientation); softmax denominators ride col 64
          via a ones column in VA
        - per (mh,tq) the 8 normalized [t,64] blocks are transposed back
          to [hd,t] via identity-matmul transposes into a fp16 PSUM tile
          that evacuates straight into UN — no broadcast matmuls, no
          partition-bounce DMA
  ACT:  most exp half-tiles (true Exp, scale folded), Q/K evacs
        (Identity+bias), usb evacs, ysb copies
  DVE:  some exp half-tiles via fp16 Schraudolph bit-trick
        (tensor_scalar fp32->int16: e^x ~ bitcast16(int16(x*1477.3/2^15
        + 15*1024 - 45)), ~3% rel err, softmax common-mode cancels),
        reciprocal of the 8 denominators, per-partition normalize
        (4x-mode tensor_scalar), unT/V/ysb copies
  Pool: weight DMAs, memsets, denominator gather (SBUF-side only:
        GPSIMD cannot touch PSUM on HW)
  SP:   fp8 x/e column-block DMAs, y output DMAs

PSUM (8 banks): 6-slot [128,512] rotation shared by att tiles and all
transients (proj accumulators, v_ps, outproj, unT) + 2 uh slots for the
flipped-AV accumulators [128,130] (h0 spans cols 0:65, h1 65:130 of one
bank; a single start/stop accumulation group per bank).

Scale bookkeeping: q' = 32q, k' = 32k, v' = 32v; exp scale 2^-15 folds
C^-0.5 and the 32*32; Wp is pre-divided by 32 on the host; y is fp16,
summed in fp32 on host.
"""

import os
import numpy as np
from collections import deque
from contextlib import ExitStack

import concourse.bass as bass
import concourse.bacc as bacc
import concourse.tile as tile
from concourse import mybir
from concourse.bass_utils import run_bass_kernel_spmd

F32 = mybir.dt.float32
F16 = mybir.dt.float16
BF16 = mybir.dt.bfloat16
F8 = mybir.dt.float8e4
I16 = mybir.dt.int16
AF = mybir.ActivationFunctionType
ALU = mybir.AluOpType
DR = mybir.MatmulPerfMode.DoubleRow

B, T, S, C = 2, 2048, 2048, 1024
H, HD = 16, 64
NCORES = 8
HPC = 4            # heads per core
MHN = 2            # head-pairs per core
KC = C // 128      # 8 contraction tiles
KP = KC // 2       # 4 DoubleRow k-pairs
STILES = S // 128  # 16
TTILES = T // 128  # 16
TQN = 4            # t-quarters of 512
SCALE_TOT = float((C ** -0.5) / 1024.0)   # exp scale on x32-scaled q,k
A16S = float((1 << 10) / np.log(2.0) * SCALE_TOT)
B16C = float(15 * (1 << 10) - 45.0)
# exp engine per s-tile (one [128,1024] op covering both heads), indexed
# [mh*TQN + tq][s]: A = ACT (true exp), D = DVE Schraudolph. (Pool cannot
# read PSUM -- the walrus BIR verifier rejects it -- so exps stay on
# ACT/DVE and the split balances their total load.)
EXP_ENGINE = [
    "ADADADAADADADADA",  # 9A 7D per quarter
    "ADADADAADADADADA",
    "ADADADAADADADADA",
    "ADADADAADADADADA",
    "ADADADAADADADADA",
    "ADADADAADADADADA",
    "ADADADAADADADADA",
    "ADADADAADADADADA",
]

LAST_RESULTS = None
_NC_CACHE = None


def _build_nc():
    nc = bacc.Bacc()

    xT8 = nc.declare_dram_parameter("xT8", [128, KC, T], F8, isOutput=False)
    eT8 = nc.declare_dram_parameter("eT8", [128, KC, S], F8, isOutput=False)
    eD8 = nc.declare_dram_parameter("eD8", [128, KC, 2, S], F8, isOutput=False)
    Wq8 = nc.declare_dram_parameter("Wq8", [128, KC, 256], F8, isOutput=False)
    Wk8 = nc.declare_dram_parameter("Wk8", [128, KC, 256], F8, isOutput=False)
    Wv8 = nc.declare_dram_parameter("Wv8", [128, KC, 256], F8, isOutput=False)
    dWv8 = nc.declare_dram_parameter("dWv8", [128, KC, 256], F8, isOutput=False)
    b6 = nc.declare_dram_parameter("b6", [128, 6], F32, isOutput=False)
    WpT4 = nc.declare_dram_parameter("WpT4", [128, 2, C], F16, isOutput=False)
    eye = nc.declare_dram_parameter("eye", [128, 128], F16, isOutput=False)
    y = nc.declare_dram_parameter("y", [T, C], F16, isOutput=True)

    with tile.TileContext(nc) as tc, ExitStack() as ctx:
        consts = ctx.enter_context(tc.tile_pool(name="consts", bufs=1))
        wpool = ctx.enter_context(tc.tile_pool(name="wts", bufs=1))
        qkvp = ctx.enter_context(tc.tile_pool(name="qkvt", bufs=2))
        vap = ctx.enter_context(tc.tile_pool(name="vaug", bufs=2))
        epool = ctx.enter_context(tc.tile_pool(name="esb", bufs=33))
        unp = ctx.enter_context(tc.tile_pool(name="unorm", bufs=2))
        usbp = ctx.enter_context(tc.tile_pool(name="usb", bufs=2))
        o16p = ctx.enter_context(tc.tile_pool(name="o16p", bufs=2))
        dnp = ctx.enter_context(tc.tile_pool(name="denom", bufs=2))
        ysbp = ctx.enter_context(tc.tile_pool(name="ysb", bufs=4))
        a32p = ctx.enter_context(tc.tile_pool(name="a32p", bufs=2))
        psp = ctx.enter_context(tc.tile_pool(name="ps", bufs=2, space="PSUM"))

        # ---- weights + small constants on the Pool DMA queue, x/e column
        # blocks on SP: both queues stream in parallel so the first QK
        # matmul can issue ~2us in.
        ed8 = wpool.tile([128, KC, 2, S], F8, tag="ed")
        nc.scalar.dma_start(out=ed8[:, :, 0, 3 * 512:4 * 512],
                            in_=eD8[:, :, 0, 3 * 512:4 * 512])
        wsb = {}
        for nm, dram in (("q", Wq8), ("k", Wk8)):
            t_ = wpool.tile([128, KC, 256], F8, tag=f"w{nm}", name=f"w{nm}sb")
            nc.gpsimd.dma_start(out=t_, in_=dram[:, :, :])
            wsb[nm] = t_
        b6sb = consts.tile([128, 6], F32, tag="b6", name="b6sb")
        nc.gpsimd.dma_start(out=b6sb, in_=b6[:, :])
        bsb = {"q": b6sb[:, 0:2], "k": b6sb[:, 2:4]}
        I128 = consts.tile([128, 128], F16, tag="eye", name="I128")
        nc.gpsimd.dma_start(out=I128, in_=eye[:, :])
        wv_sb = wpool.tile([128, KC, 256], F8, tag="wv", name="wvsb")
        nc.gpsimd.dma_start(out=wv_sb, in_=Wv8[:, :, :])
        wsb["v"] = wv_sb
        dwv_sb = wpool.tile([128, KC, 256], F8, tag="dwv", name="dwvsb")
        nc.gpsimd.dma_start(out=dwv_sb, in_=dWv8[:, :, :])
        # warm the ACT Exp/Identity table under the input-DMA head so the
        # first real exp doesn't eat the 1.3us table load
        warm = consts.tile([128, 1], F16, tag="warm", name="warm")
        nc.scalar.activation(warm, b6sb[:, 0:1], AF.Exp)

        # persistent per-pair tensors (fp8) with a small shared zero block
        # at the tail; the DoubleRow pair dim is synthesized with a custom
        # stride pointing the second half at the zero block.
        QT8 = [qkvp.tile([128, TQN * 512 + 512], F8, tag="qt", name=f"QT{i}")
               for i in range(MHN)]
        KT8 = [qkvp.tile([128, S + 128], F8, tag="kt", name=f"KT{i}")
               for i in range(MHN)]
        VA = [vap.tile([128, STILES, 130], F16, tag="va", name=f"va{i}")
              for i in range(MHN)]
        for i in range(MHN):
            nc.gpsimd.memset(QT8[i][:, TQN * 512:], 0.0)
            nc.gpsimd.memset(KT8[i][:, S:], 0.0)
            nc.gpsimd.memset(VA[i][:, :, 64:65], 1.0)
            nc.gpsimd.memset(VA[i][:, :, 129:130], 1.0)

        def with_zero_half(ap2d, zero_stride):
            """[P, N] AP -> [P, 2, N] AP whose second half reads the zero
            block `zero_stride` elements to the right."""
            return bass.AP(ap2d.tensor, ap2d.offset,
                           [ap2d.ap[0], [zero_stride, 2], ap2d.ap[1]])

        def pipeline2(producers, consumers):
            """Interleave two item lists so consumer i lands exactly 2
            producer-slots after producer i: P0 P1 C0 P2 C1 ... (with a
            2-deep PSUM slot rotation, alloc i waits for consumer i-2, so
            the consumer must be emitted before producer i+2)."""
            out = []
            for i, p in enumerate(producers):
                if i >= 2:
                    out.append(consumers[i - 2])
                out.append(p)
            out.extend(consumers[max(0, len(producers) - 2):])
            return out

        xt8 = wpool.tile([128, KC, T], F8, tag="xt")

        # fp8 x/e blocks stream on SP (chunk 3 rode the ACT queue above);
        # the de8 residual (V only) follows on the Pool queue
        def blk_dma(sb, dram, blk, eng=None):
            csl = slice(blk * 512, (blk + 1) * 512)
            (eng or nc.sync).dma_start(out=sb[:, :, csl], in_=dram[:, :, csl])

        blk_dma(xt8, xT8, 0)
        for blk in range(3):
            nc.sync.dma_start(out=ed8[:, :, 0, blk * 512:(blk + 1) * 512],
                              in_=eD8[:, :, 0, blk * 512:(blk + 1) * 512])
        for blk in range(1, 4):
            blk_dma(xt8, xT8, blk)
        for blk in range(4):
            nc.gpsimd.dma_start(out=ed8[:, :, 1, blk * 512:(blk + 1) * 512],
                                in_=eD8[:, :, 1, blk * 512:(blk + 1) * 512])
        wpt = wpool.tile([128, 2, C], F16, tag="wpt")
        nc.gpsimd.dma_start(out=wpt, in_=WpT4[:, :, :])

        # ---- deferred-work generators ----------------------------------
        def q_work(mh, tq):
            """Q^T projection for one t-quarter: 4 DoubleRow matmuls."""
            tqsl = slice(tq * 512, (tq + 1) * 512)
            state = {}

            for kp in range(KP):
                def mk_kp(kp=kp):
                    if kp == 0:
                        state["ps"] = psp.tile([128, 512], F32, tag="uh",
                                               bufs=2, name=f"qtps{mh}_{tq}")
                    nc.tensor.matmul(
                        state["ps"],
                        wsb["q"][:, 2 * kp:2 * kp + 2, mh * 128:(mh + 1) * 128],
                        xt8[:, 2 * kp:2 * kp + 2, tqsl],
                        start=(kp == 0), stop=(kp == KP - 1), perf_mode=DR)

                yield (110, mk_kp)

            def mk_evac():
                if tq % 2 == 1:
                    nc.vector.tensor_scalar(
                        out=QT8[mh][:, tq * 512:(tq + 1) * 512],
                        in0=state["ps"],
                        scalar1=bsb["q"][:, mh:mh + 1], scalar2=None,
                        op0=ALU.add)
                else:
                    nc.scalar.activation(
                        QT8[mh][:, tq * 512:(tq + 1) * 512], state["ps"],
                        AF.Identity, bias=bsb["q"][:, mh:mh + 1])

            yield (0, mk_evac)

        def kv_sc(mh, sc):
            """K^T for one 512-col s-chunk + V projection (fp8 DR, both
            heads per matmul) into VA. Returns (k-items, v-items)."""
            csl = slice(sc * 512, (sc + 1) * 512)
            state = {}

            def mk_kmm(kp, state=state):
                if kp == 0:
                    state["kt_ps"] = psp.tile([128, 512], F32, tag="uh",
                                              bufs=2, name=f"ktps{mh}_{sc}")
                ksl = slice(2 * kp, 2 * kp + 2)
                msl = slice(mh * 128, (mh + 1) * 128)
                nc.tensor.matmul(state["kt_ps"], wsb["k"][:, ksl, msl],
                                 ed8[:, ksl, 0, csl],
                                 start=(kp == 0), stop=(kp == KP - 1),
                                 perf_mode=DR)

            def mk_kevac():
                if sc % 2 == 0:
                    # DVE add-with-per-partition-bias: relieves ACT
                    nc.vector.tensor_scalar(
                        out=KT8[mh][:, csl], in0=state["kt_ps"],
                        scalar1=bsb["k"][:, mh:mh + 1], scalar2=None,
                        op0=ALU.add)
                else:
                    nc.scalar.activation(KT8[mh][:, csl], state["kt_ps"],
                                         AF.Identity,
                                         bias=bsb["k"][:, mh:mh + 1])

            vst = {}

            def mk_vp(jp):
                # V directly in [s, hd] layout for TWO s-tiles per PSUM
                # tile (disjoint col spans of one accumulation group):
                # lhsT = e^T k-pair with the fp8 residual correction, rhs
                # = Wv col block (both heads per matmul). The V bias is
                # folded into the host reduction.
                msl = slice(mh * 128, (mh + 1) * 128)
                v_ps = psp.tile([128, 256], F32, tag="uh", bufs=2,
                                name=f"vps{mh}_{sc}_{jp}")
                vst[jp] = v_ps
                for u in range(2):
                    s = sc * 4 + 2 * jp + u
                    ssl = slice(s * 128, (s + 1) * 128)
                    for g, (slot, rhs) in enumerate(
                            ((0, wsb["v"]), (1, wsb["v"]), (0, dwv_sb))):
                        for kp in range(KP):
                            nc.tensor.matmul(
                                v_ps[:, u * 128:(u + 1) * 128],
                                ed8[:, 2 * kp:2 * kp + 2, slot, ssl],
                                rhs[:, 2 * kp:2 * kp + 2, msl],
                                start=(u == 0 and g == 0 and kp == 0),
                                stop=(u == 1 and g == 2 and kp == KP - 1),
                                perf_mode=DR)

            def mk_ve(jp):
                s = sc * 4 + 2 * jp
                # strided dst: per s-tile, h0 -> cols 0:64, h1 -> 65:129
                dst = bass.AP(VA[mh].tensor, VA[mh].offset + s * 130,
                              [VA[mh].ap[0], [130, 2], [65, 2], [1, 64]])
                src2 = bass.AP(vst[jp].tensor, vst[jp].offset,
                               [vst[jp].ap[0], [128, 2], [64, 2], [1, 64]])
                nc.vector.tensor_copy(dst, src2)

            def gen_k():
                for kp in range(KP):
                    yield (110, (lambda kp=kp: mk_kmm(kp)))
                yield (0, mk_kevac)

            def gen_v():
                # produce/evac split with a 2-pop lag so the DVE copy
                # never reaches its FIFO head before the PE matmuls ran
                for w in pipeline2(
                        [(650, (lambda j=j: mk_vp(j))) for j in range(2)],
                        [(0, (lambda j=j: mk_ve(j))) for j in range(2)]):
                    yield w

            return gen_k(), gen_v()

        UN = [unp.tile([128, T], F16, tag="un", name=f"UN{i}")
              for i in range(MHN)]

        y_r = y.rearrange("(tt p) o -> tt p o", p=128)

        def av_work(mh, tq, ejs, ptag="uh", pbufs=2):
            """Flipped AV for one t-quarter: 4 accumulation groups (one
            per 128-row t-subtile; h0/h1 share the bank in disjoint col
            spans), then denominators -> reciprocal -> per-partition
            normalize (DVE 4x) -> identity transposes -> one UN evac."""
            qsl = slice(tq * 512, (tq + 1) * 512)
            state = {}

            def mk_group(j):
                uh = psp.tile([128, 130], F32, tag=ptag, bufs=pbufs,
                              name=f"uh{mh}_{tq}_{j}")
                state[j] = uh
                for h in range(2):
                    lsl = slice(h * 512 + j * 128, h * 512 + (j + 1) * 128)
                    osl = slice(h * 65, (h + 1) * 65)
                    for s in range(STILES):
                        nc.tensor.matmul(
                            uh[:, osl], ejs[s][:, lsl], VA[mh][:, s, osl],
                            start=(h == 0 and s == 0),
                            stop=(h == 1 and s == STILES - 1))

            def mk_gevac(j):
                if j == 0:
                    state["usb"] = usbp.tile([128, 4, 130], F16, tag="usb",
                                             name=f"usb{mh}_{tq}")
                if j % 2 == 0:
                    nc.vector.tensor_copy(state["usb"][:, j, :], state[j])
                else:
                    nc.scalar.copy(state["usb"][:, j, :], state[j])

            def mk_den():
                usb = state["usb"]
                den = dnp.tile([128, 8], F32, tag="den", name=f"den{mh}_{tq}")
                # cols 64 and 129 of each j-row: index 2*j + h
                src = bass.AP(usb.tensor, usb.offset + 64,
                              [usb.ap[0], [130, 4], [65, 2]])
                nc.gpsimd.tensor_copy(den, src)
                rden = dnp.tile([128, 8], F32, tag="rden",
                                name=f"rden{mh}_{tq}")
                nc.vector.reciprocal_approx_fast(rden, den)
                state["rden"] = rden

            def mk_norm(h):
                usb, rden = state["usb"], state["rden"]
                if h == 0:
                    state["o16"] = o16p.tile([128, 8, 64], F16, tag="o16",
                                             name=f"o16_{mh}_{tq}")
                o16 = state["o16"]
                for j in range(4):
                    # all-SBUF op: runs on the otherwise-idle Pool engine
                    nc.gpsimd.tensor_scalar(
                        out=o16[:, 2 * j + h, :],
                        in0=usb[:, j, h * 65:h * 65 + 64],
                        scalar1=rden[:, 2 * j + h:2 * j + h + 1],
                        scalar2=None, op0=ALU.mult)

            def mk_tr(h):
                if h == 0:
                    state["unT"] = psp.tile([128, 512], F16, tag=ptag,
                                            bufs=pbufs, name=f"unT{mh}_{tq}")
                unT = state["unT"]
                o16 = state["o16"]
                for j in range(4):
                    nc.tensor.matmul(
                        unT[h * 64:(h + 1) * 64, j * 128:(j + 1) * 128],
                        o16[:, 2 * j + h, :], I128,
                        is_transpose=True, start=(j == 0), stop=(j == 3))

            def mk_evac():
                nc.vector.tensor_copy(UN[mh][:, qsl], state["unT"])

            # software-pipelined item order: every consumer trails its
            # producer by >= 2 pops so engine FIFOs never head-of-line
            # block on in-flight PE work.
            yield (870, (lambda: mk_group(0)), mh)
            yield (870, (lambda: mk_group(1)), mh)
            yield (0, (lambda: mk_gevac(0)))
            yield (0, (lambda: mk_gevac(1)))
            yield (870, (lambda: mk_group(2)), mh)
            yield (870, (lambda: mk_group(3)), mh)
            yield (0, (lambda: mk_gevac(2)))
            yield (0, (lambda: mk_gevac(3)))
            yield (0, mk_den)
            yield (0, (lambda: mk_norm(0)))
            yield (0, (lambda: mk_norm(1)))
            yield (220, (lambda: mk_tr(0)))
            yield (220, (lambda: mk_tr(1)))
            yield (0, mk_evac)

        def av_tail_work(mh, tq, ejs):
            """Drain-optimized fused AV + outproj for the final quarter:
            per-t-subtile chains (group -> evac -> recip -> normalize ->
            transpose -> UN evac -> outproj matmuls -> ysb -> y DMA)
            interleaved so each subtile's non-PE suffix hides under the
            next subtile's PE work. Runs unpaced at drain in the idle
            att2 banks."""
            qsl = slice(tq * 512, (tq + 1) * 512)
            st = {}

            def mk_group(j):
                uh = psp.tile([128, 130], F32, tag="att2", bufs=3,
                              name=f"uht{mh}_{tq}_{j}")
                st[j] = uh
                for h in range(2):
                    lsl = slice(h * 512 + j * 128, h * 512 + (j + 1) * 128)
                    osl = slice(h * 65, (h + 1) * 65)
                    for s in range(STILES):
                        nc.tensor.matmul(
                            uh[:, osl], ejs[s][:, lsl], VA[mh][:, s, osl],
                            start=(h == 0 and s == 0),
                            stop=(h == 1 and s == STILES - 1))

            def mk_gevac(j):
                if j == 0:
                    st["usb"] = usbp.tile([128, 4, 130], F16, tag="usb",
                                          name=f"usbt{mh}_{tq}")
                if j % 2 == 0:
                    nc.vector.tensor_copy(st["usb"][:, j, :], st[j])
                else:
                    nc.scalar.copy(st["usb"][:, j, :], st[j])

            def mk_dnt(j):
                usb = st["usb"]
                den = dnp.tile([128, 2], F32, tag="dent", bufs=2,
                               name=f"dent{mh}_{tq}_{j}")
                src = bass.AP(usb.tensor, usb.offset + j * 130 + 64,
                              [usb.ap[0], [65, 2]])
                nc.gpsimd.tensor_copy(den, src)
                rden = dnp.tile([128, 2], F32, tag="rdent", bufs=2,
                                name=f"rdent{mh}_{tq}_{j}")
                nc.vector.reciprocal_approx_fast(rden, den)
                st[("r", j)] = rden
                if j == 0:
                    st["o16"] = o16p.tile([128, 8, 64], F16, tag="o16",
                                          name=f"o16t{mh}_{tq}")
                    st["unT"] = psp.tile([128, 512], F16, tag="uh",
                                         bufs=2, name=f"unTt{mh}_{tq}")
                for h in range(2):
                    nc.vector.tensor_scalar(
                        out=st["o16"][:, 2 * j + h, :],
                        in0=usb[:, j, h * 65:h * 65 + 64],
                        scalar1=rden[:, h:h + 1],
                        scalar2=None, op0=ALU.mult)

            def mk_trv(j):
                for h in range(2):
                    nc.tensor.matmul(
                        st["unT"][h * 64:(h + 1) * 64,
                                  j * 128:(j + 1) * 128],
                        st["o16"][:, 2 * j + h, :], I128,
                        is_transpose=True, start=True, stop=True)
                tsl = slice(tq * 512 + j * 128, tq * 512 + (j + 1) * 128)
                nc.vector.tensor_copy(UN[mh][:, tsl],
                                      st["unT"][:, j * 128:(j + 1) * 128])

            def mk_m(j, n):
                t = tq * 4 + j
                y_ps = psp.tile([128, 512], F32, tag="att2", bufs=3,
                                name=f"ypst{t}_{n}")
                st[("y", j, n)] = y_ps
                nsl = slice(n * 512, (n + 1) * 512)
                tsl = slice(t * 128, (t + 1) * 128)
                for m in range(MHN):
                    nc.tensor.matmul(y_ps, UN[m][:, tsl], wpt[:, m, nsl],
                                     start=(m == 0), stop=(m == MHN - 1))

            def mk_c(j, n):
                t = tq * 4 + j
                if n == 0:
                    st[("ysb", j)] = ysbp.tile([128, 1024], F16, tag="ysb",
                                               name=f"ysbt{t}")
                if (j + n) % 2 == 1:
                    nc.scalar.copy(st[("ysb", j)][:, n * 512:(n + 1) * 512],
                                   st[("y", j, n)])
                else:
                    nc.vector.tensor_copy(
                        st[("ysb", j)][:, n * 512:(n + 1) * 512],
                        st[("y", j, n)])
                eng = nc.sync if (2 * j + n) % 2 == 0 else nc.gpsimd
                eng.dma_start(out=y_r[t][:, n * 512:(n + 1) * 512],
                              in_=st[("ysb", j)][:, n * 512:(n + 1) * 512])

            def g(fn, *a):
                return (0, (lambda: fn(*a)))

            yield g(mk_group, 0)
            yield g(mk_group, 1)
            yield g(mk_gevac, 0)
            yield g(mk_dnt, 0)
            yield g(mk_trv, 0)
            yield g(mk_gevac, 1)
            yield g(mk_m, 0, 0)
            yield g(mk_m, 0, 1)
            yield g(mk_group, 2)
            yield g(mk_dnt, 1)
            yield g(mk_trv, 1)
            yield g(mk_gevac, 2)
            yield g(mk_m, 1, 0)
            yield g(mk_m, 1, 1)
            yield g(mk_c, 0, 0)
            yield g(mk_c, 0, 1)
            yield g(mk_group, 3)
            yield g(mk_dnt, 2)
            yield g(mk_trv, 2)
            yield g(mk_gevac, 3)
            yield g(mk_m, 2, 0)
            yield g(mk_m, 2, 1)
            yield g(mk_c, 1, 0)
            yield g(mk_c, 1, 1)
            yield g(mk_dnt, 3)
            yield g(mk_trv, 3)
            yield g(mk_m, 3, 0)
            yield g(mk_m, 3, 1)
            yield g(mk_c, 2, 0)
            yield g(mk_c, 2, 1)
            yield g(mk_c, 3, 0)
            yield g(mk_c, 3, 1)

        def outproj_work(tq, ptag="uh", pbufs=2):
            """Partial out-projection, pipelined across the quarter's 8
            half-units: matmul items lead their ACT/DVE evac items by 2
            slots, y DMA fires once both halves of a t-tile are copied."""
            yps = {}
            ysb = {}

            def mk_mm(u):
                j, n = u // 2, u % 2
                t = tq * 4 + j
                y_ps = psp.tile([128, 512], F32, tag=ptag, bufs=pbufs,
                                name=f"yps{t}_{n}")
                yps[u] = y_ps
                nsl = slice(n * 512, (n + 1) * 512)
                tsl = slice(t * 128, (t + 1) * 128)
                for mh in range(MHN):
                    nc.tensor.matmul(
                        y_ps, UN[mh][:, tsl], wpt[:, mh, nsl],
                        start=(mh == 0), stop=(mh == MHN - 1))

            def mk_cp(u):
                j, n = u // 2, u % 2
                t = tq * 4 + j
                if n == 0:
                    ysb[j] = ysbp.tile([128, 1024], F16, tag="ysb",
                                       name=f"ysb{t}")
                if (j + n) % 2 == 0:
                    nc.scalar.copy(ysb[j][:, n * 512:(n + 1) * 512], yps[u])
                else:
                    nc.vector.tensor_copy(ysb[j][:, n * 512:(n + 1) * 512],
                                          yps[u])
                if n == 1:
                    eng = nc.sync if j % 2 == 0 else nc.gpsimd
                    eng.dma_start(out=y_r[t], in_=ysb[j])

            for w in pipeline2(
                    [(430, (lambda u=u: mk_mm(u))) for u in range(8)],
                    [(0, (lambda u=u: mk_cp(u))) for u in range(8)]):
                yield w

        def attention(mh, fast, kq, vq, qq, credit):
            """Exp-bound attention; per s-tile: 2 zero-half DR QK matmuls
            into one [128,1024] att tile, one merged exp (ACT or DVE),
            and deferred items into the PE slack. AV is deferred
            wholesale into the next quarter via av_work."""
            def drain_k(upto_chunk):
                while kq[mh] and kq[mh][0][0] <= upto_chunk:
                    _, (c, fn) = kq[mh].popleft()
                    fn()
                    credit[0] -= c

            def drain_q(upto_tq):
                while qq[mh] and qq[mh][0][0] <= upto_tq:
                    _, (c, fn) = qq[mh].popleft()
                    fn()
                    credit[0] -= c

            def drain_other(upto_chunk):
                om = 1 - mh
                while qq[om] and qq[om][0][0] <= 0:
                    _, (c, fn) = qq[om].popleft()
                    fn()
                    credit[0] -= c
                while kq[om] and kq[om][0][0] <= upto_chunk:
                    _, (c, fn) = kq[om].popleft()
                    fn()
                    credit[0] -= c

            for tq in range(TQN):
                drain_q(tq)
                ejs = []
                for s in range(STILES):
                    drain_k(min(3, (s + 2) // 4))
                    if s == 8:
                        drain_q(min(3, tq + 1))
                    # during the last quarter of this head-pair, pre-drain
                    # the other pair's Q(tq0)/K producers so their evacs
                    # are not stuck behind the transition's exp backlog
                    if tq == TQN - 1 and s >= 6 and s % 2 == 0:
                        drain_other((s - 6) // 2)
                    ssl = slice(s * 128, (s + 1) * 128)
                    qsl_ = slice(tq * 512, (tq + 1) * 512)
                    ej = epool.tile([128, 1024], F16, tag="e",
                                    name=f"e_{mh}_{tq}_{s}")
                    ejs.append(ej)
                    eng = EXP_ENGINE[mh * TQN + tq][s]
                    att = psp.tile([128, 1024], F32, tag="att2", bufs=3,
                                   name=f"att_{mh}_{tq}_{s}")
                    for h in range(2):
                        hsl = slice(h * 64, (h + 1) * 64)
                        nc.tensor.matmul(
                            att[:, h * 512:(h + 1) * 512],
                            with_zero_half(KT8[mh][hsl, ssl], S - s * 128),
                            with_zero_half(QT8[mh][hsl, qsl_],
                                           TQN * 512 - tq * 512),
                            start=True, stop=True, perf_mode=DR)
                    if eng == "A":
                        nc.scalar.activation(ej, att, AF.Exp,
                                             scale=SCALE_TOT)
                    elif eng == "D":
                        nc.vector.tensor_scalar(
                            out=ej.bitcast(I16), in0=att,
                            scalar1=A16S, scalar2=B16C,
                            op0=ALU.mult, op1=ALU.add)
                    else:
                        # Pool Schraudolph directly from PSUM. CoreSim
                        # allows it; validated on HW (if the GPSIMD really
                        # cannot read PSUM this produces garbage and the
                        # rel-err check catches it).
                        nc.gpsimd.tensor_scalar(
                            out=ej.bitcast(I16), in0=att,
                            scalar1=A16S, scalar2=B16C,
                            op0=ALU.mult, op1=ALU.add)
                    # leaky-bucket pops: deferred PE work drains at
                    # ~560ns per s-tile; a heavy item waits until enough
                    # credit accrues so the PE FIFO never front-loads
                    # ahead of the QKs that feed the exps. av group items
                    # are gated on that head-pair's V production.
                    credit[0] = min(credit[0] + 640, 2200)
                    n = 0
                    while n < 6:
                        if fast:
                            meta = fast[0][2] if len(fast[0]) > 2 else None
                            q = (vq[meta] if meta is not None and vq[meta]
                                 else fast)
                        elif vq[mh]:
                            q = vq[mh]
                        elif vq[1 - mh]:
                            q = vq[1 - mh]
                        elif qq[1 - mh]:
                            q = qq[1 - mh]
                        elif kq[1 - mh]:
                            q = kq[1 - mh]
                        else:
                            break
                        head = q[0]
                        c, fn = head[:2] if callable(head[1]) else head[1]
                        if c > credit[0]:
                            break
                        q.popleft()
                        fn()
                        credit[0] -= c
                        n += 1
                if mh == MHN - 1 and tq == TQN - 1:
                    fast.extend(av_tail_work(mh, tq, ejs))
                else:
                    fast.extend(av_work(mh, tq, ejs))
                    if mh == MHN - 1:
                        fast.extend(outproj_work(tq))
            return fast

        # ---- schedule: eager Q(mh0,tq0) + KV(mh0,sc0); everything else
        # deferred into the attention pops. Items are emitted in pop order
        # (engine FIFOs execute in program order). Hard emission-order
        # invariants (the tile framework orders deps by program order):
        #   - a kv chunk's K-evac before the attention s-tiles reading it
        #     (kq deadline drain in the s-loop)
        #   - q_work(mh,tq) before quarter (mh,tq)'s s-loop (qq drain)
        #   - all of vq[mh] before av_work(mh,*) group items (av gate)
        for _c, w in q_work(0, 0):
            w()
        k0, v0 = kv_sc(0, 0)
        for _c, w in k0:
            w()
        kq = {0: deque(), 1: deque()}
        vq = {0: deque(), 1: deque()}
        qq = {0: deque(), 1: deque()}
        kv0 = [kv_sc(0, sc) for sc in (1, 2, 3)]
        for sc in (1, 2, 3):
            kq[0].extend((sc, it) for it in kv0[sc - 1][0])
        vq[0].extend(v0)
        for sc in (1, 2, 3):
            vq[0].extend(kv0[sc - 1][1])
        for tq in (1, 2, 3):
            qq[0].extend((tq, it) for it in q_work(0, tq))
        for tq in range(TQN):
            qq[1].extend((tq, it) for it in q_work(1, tq))
        for sc in range(4):
            gk, gv = kv_sc(1, sc)
            kq[1].extend((sc, it) for it in gk)
            vq[1].extend(gv)
        fast = deque()
        credit = [0]
        fast = attention(0, fast, kq, vq, qq, credit)
        fast = attention(1, fast, kq, vq, qq, credit)
        while fast:
            fast.popleft()[1]()


    nc.compile()
    return nc


def _get_nc():
    global _NC_CACHE
    if _NC_CACHE is None:
        _NC_CACHE = _build_nc()
    return _NC_CACHE


def make_in_maps(e, x, Wq, bq, Wk, bk, Wv, bv, Wp):
    import ml_dtypes
    NF8 = ml_dtypes.float8_e4m3

    e = np.asarray(e, dtype=np.float32)
    x = np.asarray(x, dtype=np.float32)
    Wq, bq = np.asarray(Wq, np.float32), np.asarray(bq, np.float32)
    Wk, bk = np.asarray(Wk, np.float32), np.asarray(bk, np.float32)
    Wv, bv = np.asarray(Wv, np.float32), np.asarray(bv, np.float32)
    Wp = np.asarray(Wp, np.float32)

    def swiz(a2d, dt):  # [C, N] -> [128, KC, N] partition-major
        Cd, N = a2d.shape
        return np.ascontiguousarray(
            a2d.reshape(KC, 128, N).transpose(1, 0, 2).astype(dt))

    xTs = [swiz(x[b].T, NF8) for b in range(B)]
    eTs = [swiz(e[b].T, NF8) for b in range(B)]
    # fp8 residual pair for the V projection: slot 0 = fp8(e), slot 1 =
    # fp8(e - fp32(fp8(e)))
    eD8s = []
    for b in range(B):
        a = swiz(e[b].T, np.float32)
        a8 = a.astype(NF8)
        d8 = (a - a8.astype(np.float32)).astype(NF8)
        eD8s.append(np.ascontiguousarray(np.stack([a8, d8], axis=2)))
    eye = np.eye(128, dtype=np.float16)
    in_maps = []
    for c in range(NCORES):
        b = c // 4
        h0 = (c % 4) * HPC
        cs = h0 * HD
        w8 = {}
        for nm, W, dt in (("Wq8", Wq, NF8), ("Wk8", Wk, NF8)):
            w8[nm] = swiz(W[h0:h0 + HPC].transpose(1, 0, 2)
                          .reshape(C, HPC * HD) * 32.0, dt)
        wvf = swiz(Wv[h0:h0 + HPC].transpose(1, 0, 2)
                   .reshape(C, HPC * HD) * 32.0, np.float32)
        w8["Wv8"] = wvf.astype(NF8)
        w8["dWv8"] = (wvf - w8["Wv8"].astype(np.float32)).astype(NF8)
        b6 = np.stack([bq[h0:h0 + HPC].reshape(2, 128),
                       bk[h0:h0 + HPC].reshape(2, 128),
                       bk[h0:h0 + HPC].reshape(2, 128)]) * 32.0  # slot 3 unused
        b6 = np.ascontiguousarray(
            b6.reshape(6, 128).T.astype(np.float32))      # [128, 6]
        wpt = np.ascontiguousarray(
            (Wp[:, cs:cs + HPC * HD].T / 32.0).astype(np.float16)
            .reshape(2, 128, C).transpose(1, 0, 2))       # [128, 2, C]
        in_maps.append({
            "xT8": xTs[b], "eT8": eTs[b], "eD8": eD8s[b],
            "Wq8": w8["Wq8"], "Wk8": w8["Wk8"],
            "Wv8": w8["Wv8"], "dWv8": w8["dWv8"],
            "b6": b6, "WpT4": wpt, "eye": eye,
        })
    return in_maps


def kernel(e, x, Wq, bq, Wk, bk, Wv, bv, Wp):
    global LAST_RESULTS
    nc = _get_nc()
    in_maps = make_in_maps(e, x, Wq, bq, Wk, bk, Wv, bv, Wp)
    res = run_bass_kernel_spmd(
        nc, in_maps, list(range(NCORES)),
        trace=bool(os.environ.get("BASS_TRACE")),
    )
    LAST_RESULTS = res
    # device computes attention over bias-free V; since softmax weights sum
    # to 1, the V-bias contribution to y is the constant row bv @ Wp^T,
    # added here during the partial reduction.
    bvf = np.asarray(bv, np.float32)
    Wpf = np.asarray(Wp, np.float32)
    out = np.zeros((B, T, C), dtype=np.float32)
    for c in range(NCORES):
        h0 = (c % 4) * HPC
        cs = h0 * HD
        yb = bvf[h0:h0 + HPC].reshape(-1) @ Wpf[:, cs:cs + HPC * HD].T
        out[c // 4] += res.results[c]["y"].astype(np.float32) + yb[None, :]
    return out
